# revision 2
# baseline (speedup 1.0000x reference)
"""MoE router (NoisyKGate) Trainium2 Bass kernel.

Computes, for x [B,S,D], W [D,E], b [E], k=8:
    s = sigmoid(x @ W + b)            # [B,S,E]
    g_i, idx = top_k(s, 8)            # [B,S,8]
    g = g_i / sum(g_i)
Returns (g, idx_int32, s).

Strategy: data-parallel over tokens across 8 NeuronCores. Each core gets a
2048-token shard of x, pre-transposed on the host to xT [D, 2048] so the
contraction dim D sits on SBUF partitions. Per 512-token quarter, one big DMA
loads xT[:, quarter]; 32 K-chunks x 4 token-tiles of fp32 matmuls accumulate
scores [128 tok, 64 experts] in PSUM (one bank per token-tile, bias folded in
via a rank-1 ones x b matmul); ACT applies sigmoid PSUM->SBUF; the DVE Max8 /
MaxIndex8 instructions produce top-8 values+indices per token; a batched
reciprocal-normalize finishes the gates.
"""

import os

import numpy as np

B, S, D, E, TOPK = 4, 4096, 4096, 64, 8
N_CORES = 8
P = 128
TOK_PER_CORE = (B * S) // N_CORES  # 2048
KD = D // P  # 32 contraction chunks

# tokens are processed in groups ("quarters") of TOK_PER_Q; each group is
# TILES_PER_Q psum tiles of 128 tokens
N_GROUPS = 4
TOK_PER_Q = TOK_PER_CORE // N_GROUPS  # 512
TILES_PER_Q = TOK_PER_Q // P  # 4
CHUNKS = TOK_PER_CORE // P  # 16 (column-chunks of the output views)

_CACHE = {}

LAST_RESULTS = None  # BassKernelResults of the most recent run (for test.py)


def _build_kernel(tok_per_core=TOK_PER_CORE, d=D, n_groups=N_GROUPS):
    """Build the per-core Bass module. Same program on all 8 cores."""
    import concourse.bacc as bacc
    import concourse.mybir as mybir
    from concourse.tile import TileContext

    f32 = mybir.dt.float32
    kd = d // P
    tok_per_q = tok_per_core // n_groups
    tiles_per_q = tok_per_q // P

    nc = bacc.Bacc("TRN2", target_bir_lowering=False, debug=False)

    xT = nc.dram_tensor("xT", [d, tok_per_core], f32, kind="ExternalInput")
    Wt = nc.dram_tensor("Wt", [P, kd, E], f32, kind="ExternalInput")
    bt = nc.dram_tensor("bt", [1, E], f32, kind="ExternalInput")
    s_out = nc.dram_tensor("s_out", [tok_per_core, E], f32, kind="ExternalOutput")
    g_out = nc.dram_tensor("g_out", [tok_per_core, TOPK], f32, kind="ExternalOutput")
    i_out = nc.dram_tensor(
        "i_out", [tok_per_core, TOPK], mybir.dt.int32, kind="ExternalOutput"
    )

    # token t = p*CHUNKS + c lives at SBUF partition p, column-chunk c
    chunks = tok_per_core // P
    s_view = s_out.ap().rearrange("(p c) e -> p c e", p=P)  # [128, chunks, 64]
    g_view = g_out.ap().rearrange("(p c) k -> p c k", p=P)
    i_view = i_out.ap().rearrange("(p c) k -> p c k", p=P)
    # xT columns are (c, p) so that matmul tile i covers columns i*128..i*128+127
    xq_view = xT.ap().rearrange("(ko p) t -> p ko t", p=P)  # [128, kd, tok]

    with TileContext(nc) as tc:
        with (
            tc.tile_pool(name="const", bufs=1) as cpool,
            tc.tile_pool(name="xq", bufs=2) as xpool,
            tc.tile_pool(name="psum", bufs=8, space="PSUM") as ppool,
            tc.tile_pool(name="post", bufs=2) as spool,
        ):
            W_sb = cpool.tile([P, kd, E], f32)
            nc.sync.dma_start(W_sb[:], Wt.ap())
            b_sb = cpool.tile([1, E], f32)
            nc.sync.dma_start(b_sb[:], bt.ap())
            ones_sb = cpool.tile([1, P], f32)
            nc.vector.memset(ones_sb[:], 1.0)

            for q in range(n_groups):
                # one big DMA for this token group: [128, kd, tok_per_q]
                xq = xpool.tile([P, kd, tok_per_q], f32, tag="xq")
                nc.sync.dma_start(
                    xq[:], xq_view[:, :, q * tok_per_q : (q + 1) * tok_per_q]
                )

                psums = []
                for i in range(tiles_per_q):
                    ps = ppool.tile([P, E], f32, tag="ps", name=f"ps_{q}_{i}")
                    psums.append(ps)
                # bias: psum[i] = ones[128] x b[64]  (rank-1, starts the group)
                for i in range(tiles_per_q):
                    nc.tensor.matmul(
                        psums[i][:], ones_sb[:], b_sb[:], start=True, stop=False
                    )
                for ko in range(kd):
                    for i in range(tiles_per_q):
                        nc.tensor.matmul(
                            psums[i][:],
                            xq[:, ko, i * P : (i + 1) * P],  # lhsT [K=128, M=128 tok]
                            W_sb[:, ko, :],  # rhs  [K=128, N=64 experts]
                            start=False,
                            stop=(ko == kd - 1),
                        )

                # post-process: sigmoid + top-8 + normalize
                s_q = spool.tile([P, tiles_per_q, E], f32, tag="s_q")
                gmax = spool.tile([P, tiles_per_q, TOPK], f32, tag="gmax")
                gidx = spool.tile([P, tiles_per_q, TOPK], mybir.dt.uint32, tag="gidx")
                gsum = spool.tile([P, tiles_per_q], f32, tag="gsum")
                grec = spool.tile([P, tiles_per_q], f32, tag="grec")
                gnrm = spool.tile([P, tiles_per_q, TOPK], f32, tag="gnrm")
                for i in range(tiles_per_q):
                    nc.scalar.activation(
                        s_q[:, i], psums[i][:], mybir.ActivationFunctionType.Sigmoid
                    )
                    nc.vector.max(out=gmax[:, i], in_=s_q[:, i])
                    nc.vector.max_index(
                        out=gidx[:, i], in_max=gmax[:, i], in_values=s_q[:, i]
                    )
                nc.vector.reduce_sum(gsum[:], gmax[:], axis=mybir.AxisListType.X)
                nc.vector.reciprocal(grec[:], gsum[:])
                nc.vector.tensor_mul(
                    gnrm[:],
                    gmax[:],
                    grec[:, :, None].to_broadcast([P, tiles_per_q, TOPK]),
                )

                c0 = q * tiles_per_q
                c1 = c0 + tiles_per_q
                nc.sync.dma_start(s_view[:, c0:c1, :], s_q[:])
                nc.sync.dma_start(g_view[:, c0:c1, :], gnrm[:])
                nc.sync.dma_start(
                    i_view[:, c0:c1, :], gidx[:].bitcast(mybir.dt.int32)
                )

    nc.compile()
    return nc


def _get_nc():
    key = "main"
    if key not in _CACHE:
        _CACHE[key] = _build_kernel()
    return _CACHE[key]


def _prep_inputs(x, W, b):
    """Host-side shard + transpose. Returns per-core in_maps."""
    x = np.ascontiguousarray(np.asarray(x, dtype=np.float32))
    W = np.ascontiguousarray(np.asarray(W, dtype=np.float32))
    b = np.ascontiguousarray(np.asarray(b, dtype=np.float32))

    x_flat = x.reshape(N_CORES, P, CHUNKS, D)  # core j, partition p, chunk c
    # -> [core, D, chunk, partition]; column index of xT = c*128 + p
    xT_all = np.ascontiguousarray(x_flat.transpose(0, 3, 2, 1))

    W_sb = np.ascontiguousarray(
        W.reshape(KD, P, E).transpose(1, 0, 2)
    )  # [128, 32, 64]
    b_sb = np.ascontiguousarray(b.reshape(1, E))

    in_maps = []
    for j in range(N_CORES):
        in_maps.append(
            {
                "xT": xT_all[j].reshape(D, TOK_PER_CORE),
                "Wt": W_sb,
                "bt": b_sb,
            }
        )
    return in_maps


def kernel(x, W, b, k):
    global LAST_RESULTS
    from concourse.bass_utils import run_bass_kernel_spmd

    k = int(np.asarray(k))
    assert k == TOPK, f"kernel hardcodes k=8, got {k}"
    assert tuple(np.asarray(x).shape) == (B, S, D)

    nc = _get_nc()
    in_maps = _prep_inputs(x, W, b)

    res = run_bass_kernel_spmd(
        nc,
        in_maps,
        core_ids=list(range(N_CORES)),
        trace=bool(int(os.environ.get("KERNEL_TRACE", "0"))),
    )
    LAST_RESULTS = res

    g = np.concatenate([r["g_out"] for r in res.results], axis=0)
    idx = np.concatenate([r["i_out"] for r in res.results], axis=0)
    s = np.concatenate([r["s_out"] for r in res.results], axis=0)
    return (
        g.reshape(B, S, TOPK),
        idx.reshape(B, S, TOPK).astype(np.int32),
        s.reshape(B, S, E),
    )


# revision 4
# speedup vs baseline: 1.7926x; 1.7926x over previous
"""MoE router (NoisyKGate) Trainium2 Bass kernel.

Computes, for x [B,S,D], W [D,E], b [E], k=8:
    s = sigmoid(x @ W + b)            # [B,S,E]
    g_i, idx = top_k(s, 8)            # [B,S,8]
    g = g_i / sum(g_i)
Returns (g, idx_int32, s).

Strategy: data-parallel over tokens across 8 NeuronCores; each core gets a
2048-token shard. The matmul runs as an fp16 high/low 3-pass decomposition:
on the host, x = xh + 2^-12 * xl and W = wh + 2^-12 * wl with xh/xl/wh/wl all
fp16. The PE multiplies fp16 exactly (11x11-bit mantissas fit the FP22
datapath) and accumulates fp32, so
    x@W ~= xh@wh + 2^-12 * (xh@wl + xl@wh)        (error ~2^-22 relative)
which beats the HW fp32 4-pass in accuracy while streaming at 1 cycle/row
(4x faster) and halving input DMA bytes.

Layout: W-chunks are the stationary operand [128, 64]; x arrives host-
transposed as xT [D, tok] so each matmul streams 512 tokens. Scores
accumulate in PSUM [64 experts, 512 tok]; one DVE op combines the hi/lo
accumulators; ACT applies sigmoid with the per-partition expert bias; PE
transposes back to [128 tok, 64 experts]; DVE Max8/MaxIndex8 produce the
top-8 values+indices per token; a batched reciprocal-normalize finishes.
"""

import os

import numpy as np

B, S, D, E, TOPK = 4, 4096, 4096, 64, 8
N_CORES = 8
P = 128
TOK_PER_CORE = (B * S) // N_CORES  # 2048
KD = D // P  # 32 contraction chunks
N_GROUPS = 4
TOK_PER_Q = TOK_PER_CORE // N_GROUPS  # 512
TILES_PER_Q = TOK_PER_Q // P  # 4
CHUNKS = TOK_PER_CORE // P  # 16

LO_SCALE = float(2**-12)

_CACHE = {}
LAST_RESULTS = None


def _build_kernel(tok_per_core=TOK_PER_CORE, d=D, n_groups=N_GROUPS):
    import concourse.bacc as bacc
    import concourse.mybir as mybir
    from concourse.masks import make_identity
    from concourse.tile import TileContext

    f32 = mybir.dt.float32
    f16 = mybir.dt.float16
    kd = d // P
    tok_per_q = tok_per_core // n_groups
    tiles_per_q = tok_per_q // P
    chunks = tok_per_core // P

    nc = bacc.Bacc("TRN2", target_bir_lowering=False, debug=False)

    xh_d = nc.dram_tensor("xh", [d, tok_per_core], f16, kind="ExternalInput")
    xl_d = nc.dram_tensor("xl", [d, tok_per_core], f16, kind="ExternalInput")
    wh_d = nc.dram_tensor("wh", [P, kd, E], f16, kind="ExternalInput")
    wl_d = nc.dram_tensor("wl", [P, kd, E], f16, kind="ExternalInput")
    b_d = nc.dram_tensor("bt", [E, 1], f32, kind="ExternalInput")
    s_out = nc.dram_tensor("s_out", [tok_per_core, E], f32, kind="ExternalOutput")
    g_out = nc.dram_tensor("g_out", [tok_per_core, TOPK], f32, kind="ExternalOutput")
    i_out = nc.dram_tensor(
        "i_out", [tok_per_core, TOPK], mybir.dt.int32, kind="ExternalOutput"
    )

    # token t = p*chunks + c lives at SBUF partition p, column-chunk c
    s_view = s_out.ap().rearrange("(p c) e -> p c e", p=P)
    g_view = g_out.ap().rearrange("(p c) k -> p c k", p=P)
    i_view = i_out.ap().rearrange("(p c) k -> p c k", p=P)
    xh_view = xh_d.ap().rearrange("(ko p) t -> p ko t", p=P)
    xl_view = xl_d.ap().rearrange("(ko p) t -> p ko t", p=P)

    with TileContext(nc) as tc:
        with (
            tc.tile_pool(name="const", bufs=1) as cpool,
            tc.tile_pool(name="xq", bufs=2) as xpool,
            tc.tile_pool(name="psum", bufs=2, space="PSUM") as ppool,
            tc.tile_pool(name="post", bufs=2) as spool,
        ):
            wh_sb = cpool.tile([P, kd, E], f16)
            nc.sync.dma_start(wh_sb[:], wh_d.ap())
            wl_sb = cpool.tile([P, kd, E], f16)
            nc.sync.dma_start(wl_sb[:], wl_d.ap())
            b_sb = cpool.tile([E, 1], f32)
            nc.sync.dma_start(b_sb[:], b_d.ap())
            ident = cpool.tile([E, E], f32)
            make_identity(nc, ident[:])

            for q in range(n_groups):
                t0, t1 = q * tok_per_q, (q + 1) * tok_per_q
                xh_q = xpool.tile([P, kd, tok_per_q], f16, tag="xh_q")
                nc.sync.dma_start(xh_q[:], xh_view[:, :, t0:t1])
                xl_q = xpool.tile([P, kd, tok_per_q], f16, tag="xl_q")
                nc.sync.dma_start(xl_q[:], xl_view[:, :, t0:t1])

                psA = ppool.tile([E, tok_per_q], f32, tag="psA", name=f"psA_{q}")
                psB = ppool.tile([E, tok_per_q], f32, tag="psB", name=f"psB_{q}")
                for ko in range(kd):
                    last = ko == kd - 1
                    # HH -> psA; (LH, HL) -> psB (carry the 2^-12 factor)
                    nc.tensor.matmul(
                        psA[:], wh_sb[:, ko, :], xh_q[:, ko, :],
                        start=(ko == 0), stop=last,
                    )
                    nc.tensor.matmul(
                        psB[:], wh_sb[:, ko, :], xl_q[:, ko, :],
                        start=(ko == 0), stop=False,
                    )
                    nc.tensor.matmul(
                        psB[:], wl_sb[:, ko, :], xh_q[:, ko, :],
                        start=False, stop=last,
                    )

                # logits = psA + 2^-12 * psB   [64, 512]
                # (one DVE op may read at most one PSUM operand, so bounce
                # the scaled low part through SBUF first)
                tmpB = spool.tile([E, tok_per_q], f32, tag="tmpB")
                nc.vector.tensor_scalar_mul(tmpB[:], psB[:], LO_SCALE)
                logit = spool.tile([E, tok_per_q], f32, tag="logit")
                nc.vector.tensor_add(logit[:], tmpB[:], psA[:])
                # sT = sigmoid(logits + b)   (bias is per-partition here)
                sT = spool.tile([E, tok_per_q], f32, tag="sT")
                nc.scalar.activation(
                    sT[:], logit[:], mybir.ActivationFunctionType.Sigmoid,
                    bias=b_sb[:],
                )
                # transpose back to [128 tok, 64 experts]
                psT = ppool.tile([P, tiles_per_q * E], f32, tag="psT", name=f"psT_{q}")
                for i in range(tiles_per_q):
                    nc.tensor.transpose(
                        psT[:, i * E : (i + 1) * E],
                        sT[:, i * P : (i + 1) * P],
                        ident[:],
                    )
                s_q = spool.tile([P, tiles_per_q, E], f32, tag="s_q")
                nc.vector.tensor_copy(
                    s_q[:], psT[:].rearrange("p (c e) -> p c e", c=tiles_per_q)
                )

                # top-8 + normalize
                gmax = spool.tile([P, tiles_per_q, TOPK], f32, tag="gmax")
                gidx = spool.tile([P, tiles_per_q, TOPK], mybir.dt.uint32, tag="gidx")
                gsum = spool.tile([P, tiles_per_q], f32, tag="gsum")
                grec = spool.tile([P, tiles_per_q], f32, tag="grec")
                gnrm = spool.tile([P, tiles_per_q, TOPK], f32, tag="gnrm")
                for i in range(tiles_per_q):
                    nc.vector.max(out=gmax[:, i], in_=s_q[:, i])
                    nc.vector.max_index(
                        out=gidx[:, i], in_max=gmax[:, i], in_values=s_q[:, i]
                    )
                nc.vector.reduce_sum(gsum[:], gmax[:], axis=mybir.AxisListType.X)
                nc.vector.reciprocal(grec[:], gsum[:])
                nc.vector.tensor_mul(
                    gnrm[:],
                    gmax[:],
                    grec[:, :, None].to_broadcast([P, tiles_per_q, TOPK]),
                )

                c0 = q * tiles_per_q
                c1 = c0 + tiles_per_q
                nc.sync.dma_start(s_view[:, c0:c1, :], s_q[:])
                nc.sync.dma_start(g_view[:, c0:c1, :], gnrm[:])
                nc.sync.dma_start(i_view[:, c0:c1, :], gidx[:].bitcast(mybir.dt.int32))

    nc.compile()
    return nc


def _get_nc():
    key = "main"
    if key not in _CACHE:
        _CACHE[key] = _build_kernel()
    return _CACHE[key]


def _split_hl(a32):
    """a32 (fp32) -> (hi fp16, lo fp16) with a32 ~= hi + 2^-12 * lo."""
    hi = a32.astype(np.float16)
    lo = ((a32 - hi.astype(np.float32)) * 4096.0).astype(np.float16)
    return hi, lo


def _prep_inputs(x, W, b):
    x = np.asarray(x, dtype=np.float32)
    W = np.ascontiguousarray(np.asarray(W, dtype=np.float32))
    b = np.ascontiguousarray(np.asarray(b, dtype=np.float32))

    xh, xl = _split_hl(x.reshape(N_CORES, P, CHUNKS, D))
    # -> [core, D, chunk, partition]; xT column index = c*128 + p = token p*16+c
    xh_all = np.ascontiguousarray(xh.transpose(0, 3, 2, 1))
    xl_all = np.ascontiguousarray(xl.transpose(0, 3, 2, 1))

    wh, wl = _split_hl(W)
    wh_sb = np.ascontiguousarray(wh.reshape(KD, P, E).transpose(1, 0, 2))
    wl_sb = np.ascontiguousarray(wl.reshape(KD, P, E).transpose(1, 0, 2))
    b_sb = np.ascontiguousarray(b.reshape(E, 1))

    in_maps = []
    for j in range(N_CORES):
        in_maps.append(
            {
                "xh": xh_all[j].reshape(D, TOK_PER_CORE),
                "xl": xl_all[j].reshape(D, TOK_PER_CORE),
                "wh": wh_sb,
                "wl": wl_sb,
                "bt": b_sb,
            }
        )
    return in_maps


def kernel(x, W, b, k):
    global LAST_RESULTS
    from concourse.bass_utils import run_bass_kernel_spmd

    k = int(np.asarray(k))
    assert k == TOPK, f"kernel hardcodes k=8, got {k}"
    assert tuple(np.asarray(x).shape) == (B, S, D)

    nc = _get_nc()
    in_maps = _prep_inputs(x, W, b)

    res = run_bass_kernel_spmd(
        nc,
        in_maps,
        core_ids=list(range(N_CORES)),
        trace=bool(int(os.environ.get("KERNEL_TRACE", "0"))),
    )
    LAST_RESULTS = res

    g = np.concatenate([r["g_out"] for r in res.results], axis=0)
    idx = np.concatenate([r["i_out"] for r in res.results], axis=0)
    s = np.concatenate([r["s_out"] for r in res.results], axis=0)
    return (
        g.reshape(B, S, TOPK),
        idx.reshape(B, S, TOPK).astype(np.int32),
        s.reshape(B, S, E),
    )


# revision 6
# speedup vs baseline: 2.2059x; 1.2306x over previous
"""MoE router (NoisyKGate) Trainium2 Bass kernel.

Computes, for x [B,S,D], W [D,E], b [E], k=8:
    s = sigmoid(x @ W + b)            # [B,S,E]
    g_i, idx = top_k(s, 8)            # [B,S,8]
    g = g_i / sum(g_i)
Returns (g, idx_int32, s).

Strategy: data-parallel over tokens across 8 NeuronCores; each core gets a
2048-token shard. The matmul runs as an fp16 high/low 3-pass decomposition:
on the host, x = xh + 2^-12 * xl and W = wh + 2^-12 * wl with xh/xl/wh/wl all
fp16. The PE multiplies fp16 exactly (11x11-bit mantissas fit the FP22
datapath) and accumulates fp32, so
    x@W ~= xh@wh + 2^-12 * (xh@wl + xl@wh)        (error ~2^-22 relative)
which beats the HW fp32 4-pass in accuracy while streaming at 1 cycle/row.

Layout: W-chunks are the stationary operand; x arrives host-packed in the
exact SBUF layout [group, partition, chunk, hi/lo, token] so each group is
one fully-contiguous 8.4MB DMA (64KB per partition). Matmuls are
column-tiled: two D-chunks run concurrently on PE column groups (0,0) and
(0,64), accumulating into PSUM partitions 0:64 / 64:128. The two chunk-
parity halves are summed after the PE transpose back to token-major layout,
where the expert bias is also added; ACT applies sigmoid; DVE Max8/
MaxIndex8 produce the top-8 values+indices; a batched reciprocal-normalize
finishes the gates.
"""

import os

import numpy as np

B, S, D, E, TOPK = 4, 4096, 4096, 64, 8
N_CORES = 8
P = 128
TOK_PER_CORE = (B * S) // N_CORES  # 2048
KD = D // P  # 32 contraction chunks
N_GROUPS = 4
TOK_PER_Q = TOK_PER_CORE // N_GROUPS  # 512
TILES_PER_Q = TOK_PER_Q // P  # 4
CHUNKS = TOK_PER_CORE // P  # 16

LO_SCALE = float(2**-12)

_CACHE = {}
LAST_RESULTS = None


def _build_kernel(tok_per_core=TOK_PER_CORE, d=D, n_groups=N_GROUPS):
    import concourse.bacc as bacc
    import concourse.mybir as mybir
    from concourse.masks import make_identity
    from concourse.tile import TileContext

    f32 = mybir.dt.float32
    f16 = mybir.dt.float16
    kd = d // P
    tok_per_q = tok_per_core // n_groups
    tiles_per_q = tok_per_q // P
    chunks = tok_per_core // P
    pairs = kd // 2

    nc = bacc.Bacc("TRN2", target_bir_lowering=False, debug=False)

    xc_d = nc.dram_tensor(
        "xc", [n_groups, P, kd, 2, tok_per_q], f16, kind="ExternalInput"
    )
    wh_d = nc.dram_tensor("wh", [P, kd, E], f16, kind="ExternalInput")
    wl_d = nc.dram_tensor("wl", [P, kd, E], f16, kind="ExternalInput")
    br_d = nc.dram_tensor("br", [P, E], f32, kind="ExternalInput")
    s_out = nc.dram_tensor("s_out", [tok_per_core, E], f32, kind="ExternalOutput")
    g_out = nc.dram_tensor("g_out", [tok_per_core, TOPK], f32, kind="ExternalOutput")
    i_out = nc.dram_tensor(
        "i_out", [tok_per_core, TOPK], mybir.dt.int32, kind="ExternalOutput"
    )

    # token t = p*chunks + c lives at SBUF partition p, column-chunk c
    s_view = s_out.ap().rearrange("(p c) e -> p c e", p=P)
    g_view = g_out.ap().rearrange("(p c) k -> p c k", p=P)
    i_view = i_out.ap().rearrange("(p c) k -> p c k", p=P)
    xc_ap = xc_d.ap()

    with TileContext(nc) as tc:
        with (
            tc.tile_pool(name="const", bufs=1) as cpool,
            tc.tile_pool(name="xq", bufs=2) as xpool,
            tc.tile_pool(name="psum", bufs=2, space="PSUM") as ppool,
            tc.tile_pool(name="post", bufs=2) as spool,
        ):
            wh_sb = cpool.tile([P, kd, E], f16)
            nc.sync.dma_start(wh_sb[:], wh_d.ap())
            wl_sb = cpool.tile([P, kd, E], f16)
            nc.sync.dma_start(wl_sb[:], wl_d.ap())
            br_sb = cpool.tile([P, E], f32)
            nc.sync.dma_start(br_sb[:], br_d.ap())
            ident = cpool.tile([P, P], f32)
            make_identity(nc, ident[:])

            for q in range(n_groups):
                xq = xpool.tile([P, kd, 2, tok_per_q], f16, tag="xq")
                # split the group load in two so the first matmuls can
                # start after half the bytes land
                half = kd // 2
                nc.sync.dma_start(xq[:, :half], xc_ap[q, :, :half])
                nc.sync.dma_start(xq[:, half:], xc_ap[q, :, half:])

                psA = ppool.tile([P, tok_per_q], f32, tag="psA", name=f"psA_{q}")
                psB = ppool.tile([P, tok_per_q], f32, tag="psB", name=f"psB_{q}")
                for ko2 in range(pairs):
                    k0, k1 = 2 * ko2, 2 * ko2 + 1
                    first = ko2 == 0
                    last = ko2 == pairs - 1
                    xh0, xl0 = xq[:, k0, 0, :], xq[:, k0, 1, :]
                    xh1, xl1 = xq[:, k1, 0, :], xq[:, k1, 1, :]
                    nc.tensor.matmul(
                        psA[0:64, :], wh_sb[:, k0, :], xh0,
                        tile_position=(0, 0), start=first, stop=last, skip_group_check=True,
                    )
                    nc.tensor.matmul(
                        psA[64:128, :], wh_sb[:, k1, :], xh1,
                        tile_position=(0, 64), start=first, stop=last, skip_group_check=True,
                    )
                    nc.tensor.matmul(
                        psB[0:64, :], wh_sb[:, k0, :], xl0,
                        tile_position=(0, 0), start=first, stop=False, skip_group_check=True,
                    )
                    nc.tensor.matmul(
                        psB[64:128, :], wh_sb[:, k1, :], xl1,
                        tile_position=(0, 64), start=first, stop=False, skip_group_check=True,
                    )
                    nc.tensor.matmul(
                        psB[0:64, :], wl_sb[:, k0, :], xh0,
                        tile_position=(0, 0), start=False, stop=last, skip_group_check=True,
                    )
                    nc.tensor.matmul(
                        psB[64:128, :], wl_sb[:, k1, :], xh1,
                        tile_position=(0, 64), start=False, stop=last, skip_group_check=True,
                    )

                # partial logits (chunk-parity halves stacked on partitions):
                # logitTT = psA + 2^-12 * psB    [128, 512]
                psA_sb = spool.tile([P, tok_per_q], f32, tag="psA_sb")
                nc.vector.tensor_copy(psA_sb[:], psA[:])
                logitTT = spool.tile([P, tok_per_q], f32, tag="logitTT")
                nc.vector.scalar_tensor_tensor(
                    logitTT[:], psB[:], LO_SCALE, psA_sb[:],
                    op0=mybir.AluOpType.mult, op1=mybir.AluOpType.add,
                )
                # transpose to token-major: psT [128 tok, (half, 64 expert)]
                psT = ppool.tile([P, tiles_per_q * P], f32, tag="psT", name=f"psT_{q}")
                for i in range(tiles_per_q):
                    nc.tensor.transpose(
                        psT[:, i * P : (i + 1) * P],
                        logitTT[:, i * P : (i + 1) * P],
                        ident[:],
                    )
                cc = spool.tile([P, tiles_per_q, P], f32, tag="cc")
                nc.vector.tensor_copy(
                    cc[:], psT[:].rearrange("p (c he) -> p c he", c=tiles_per_q)
                )
                # sum the parity halves, add bias, sigmoid
                logit_q = spool.tile([P, tiles_per_q, E], f32, tag="logit_q")
                nc.vector.tensor_add(logit_q[:], cc[:, :, 0:E], cc[:, :, E:P])
                nc.vector.tensor_add(
                    logit_q[:],
                    logit_q[:],
                    br_sb[:, None, :].to_broadcast([P, tiles_per_q, E]),
                )
                s_q = spool.tile([P, tiles_per_q, E], f32, tag="s_q")
                nc.scalar.activation(
                    s_q[:], logit_q[:], mybir.ActivationFunctionType.Sigmoid
                )

                # top-8 + normalize
                gmax = spool.tile([P, tiles_per_q, TOPK], f32, tag="gmax")
                gidx = spool.tile([P, tiles_per_q, TOPK], mybir.dt.uint32, tag="gidx")
                gsum = spool.tile([P, tiles_per_q], f32, tag="gsum")
                grec = spool.tile([P, tiles_per_q], f32, tag="grec")
                gnrm = spool.tile([P, tiles_per_q, TOPK], f32, tag="gnrm")
                for i in range(tiles_per_q):
                    nc.vector.max(out=gmax[:, i], in_=s_q[:, i])
                    nc.vector.max_index(
                        out=gidx[:, i], in_max=gmax[:, i], in_values=s_q[:, i]
                    )
                nc.vector.reduce_sum(gsum[:], gmax[:], axis=mybir.AxisListType.X)
                nc.vector.reciprocal(grec[:], gsum[:])
                nc.vector.tensor_mul(
                    gnrm[:],
                    gmax[:],
                    grec[:, :, None].to_broadcast([P, tiles_per_q, TOPK]),
                )

                c0 = q * tiles_per_q
                c1 = c0 + tiles_per_q
                nc.sync.dma_start(s_view[:, c0:c1, :], s_q[:])
                nc.sync.dma_start(g_view[:, c0:c1, :], gnrm[:])
                nc.sync.dma_start(i_view[:, c0:c1, :], gidx[:].bitcast(mybir.dt.int32))

    nc.compile()
    return nc


def _get_nc():
    key = "main"
    if key not in _CACHE:
        _CACHE[key] = _build_kernel()
    return _CACHE[key]


def _split_hl(a32):
    """a32 (fp32) -> (hi fp16, lo fp16) with a32 ~= hi + 2^-12 * lo."""
    hi = a32.astype(np.float16)
    lo = ((a32 - hi.astype(np.float32)) * 4096.0).astype(np.float16)
    return hi, lo


def _prep_inputs(x, W, b):
    x = np.asarray(x, dtype=np.float32)
    W = np.ascontiguousarray(np.asarray(W, dtype=np.float32))
    b = np.ascontiguousarray(np.asarray(b, dtype=np.float32))

    # token t (within a core) = p*CHUNKS + c, c = q*TILES_PER_Q + i,
    # matmul-group column index = i*128 + pp; xc[core, q, p, ko, h, i*128+pp]
    # = x_{h/l}[core, token pp*CHUNKS + q*TILES_PER_Q + i, d = ko*128 + p]
    xh, xl = _split_hl(x.reshape(N_CORES, P, N_GROUPS, TILES_PER_Q, KD, P))
    xc = np.empty((N_CORES, N_GROUPS, P, KD, 2, TILES_PER_Q, P), np.float16)
    xc[:, :, :, :, 0] = xh.transpose(0, 2, 5, 4, 3, 1)
    xc[:, :, :, :, 1] = xl.transpose(0, 2, 5, 4, 3, 1)
    xc = xc.reshape(N_CORES, N_GROUPS, P, KD, 2, TOK_PER_Q)

    wh, wl = _split_hl(W)
    wh_sb = np.ascontiguousarray(wh.reshape(KD, P, E).transpose(1, 0, 2))
    wl_sb = np.ascontiguousarray(wl.reshape(KD, P, E).transpose(1, 0, 2))
    br = np.ascontiguousarray(np.broadcast_to(b.reshape(1, E), (P, E)))

    in_maps = []
    for j in range(N_CORES):
        in_maps.append(
            {"xc": xc[j], "wh": wh_sb, "wl": wl_sb, "br": br}
        )
    return in_maps


def kernel(x, W, b, k):
    global LAST_RESULTS
    from concourse.bass_utils import run_bass_kernel_spmd

    k = int(np.asarray(k))
    assert k == TOPK, f"kernel hardcodes k=8, got {k}"
    assert tuple(np.asarray(x).shape) == (B, S, D)

    nc = _get_nc()
    in_maps = _prep_inputs(x, W, b)

    res = run_bass_kernel_spmd(
        nc,
        in_maps,
        core_ids=list(range(N_CORES)),
        trace=bool(int(os.environ.get("KERNEL_TRACE", "0"))),
    )
    LAST_RESULTS = res

    g = np.concatenate([r["g_out"] for r in res.results], axis=0)
    idx = np.concatenate([r["i_out"] for r in res.results], axis=0)
    s = np.concatenate([r["s_out"] for r in res.results], axis=0)
    return (
        g.reshape(B, S, TOPK),
        idx.reshape(B, S, TOPK).astype(np.int32),
        s.reshape(B, S, E),
    )
